# revision 1
# baseline (speedup 1.0000x reference)
"""Trainium2 Bass kernel for EnhancedTripletLoss — v2 (hybrid reduction).

Changes vs v1 (124µs baseline):
  * A custom DVE op TT_ADD_MINRED (body=Src0+Src1, accum=min) fuses the
    ||e_j||^2 addition into the block-min reduction, so DIRECT blocks need
    only the two K=128 bf16 matmul passes (the K=3 sq channel is dropped
    there): PE per tile falls from 3N to 2N + y*1024 columns.
  * y=5 OFFLOAD blocks keep the PE sq channel and drain through
    ACT convert(fp32 PSUM->fp16 SBUF) -> TT_MIN_MINRED, a second custom
    op that min-reduces a block by feeding its two halves as Src0/Src1
    (512-elem pass, ~0.72us), halving their DVE cost; direct blocks use
    TT_ADD_MINRED straight from PSUM (~1.3us) with no PE sq pass.
  * Per-tile epilogue is DVE-only; relu/sqrt/margin run once, batched,
    after the loop.
  * sq is carried as ±(sq-256) (sign trick per core, offset for fp16/bf16
    ulp); the epilogue bias becomes sqa+256.
  * Input DMAs moved off the gpsimd queue (Pool now computes).
"""

import numpy as np
import ml_dtypes

P = 128
D = 256
NCLS = 8
NCORES = 8
MARGIN = 0.3
BIGM = 1.0e30
NTERMS = 1
SQTERMS = 3
WMAIN = 1024
SQOFF = 256.0      # offset folded out of sq for fp16 precision
PADV = 60000.0     # fp16 pad for offload fold tree
Y_OFF = 5          # blocks routed through ACT+pair-min offload

BF16 = ml_dtypes.bfloat16


def _ref_add_minred(in0, in1, c0, c1, c2):
    b = in0.astype(np.float32) + in1.astype(np.float32)
    b2 = b.reshape(b.shape[0], -1)
    seed = np.asarray(c0, np.float32).reshape(-1, 1)
    acc = np.minimum(b2.min(axis=-1, keepdims=True), seed)
    return b, acc


def _ref_min_minred(in0, in1, c0, c1, c2):
    b = np.minimum(in0.astype(np.float32), in1.astype(np.float32))
    b2 = b.reshape(b.shape[0], -1)
    seed = np.asarray(c0, np.float32).reshape(-1, 1)
    acc = np.minimum(b2.min(axis=-1, keepdims=True), seed)
    return b, acc


def _register_op(name, spec):
    from concourse.dve_ops import DveOp, OPS, CUSTOM_DVE_SPECS, _SUB_OPCODE_FOR_NAME

    if name in _SUB_OPCODE_FOR_NAME:
        return next(op for op in OPS if op.name == name)
    op = DveOp(name, spec, subdim=False, uops_sha={})
    row = max(_SUB_OPCODE_FOR_NAME.values()) + 1
    assert row < 0x20
    OPS.append(op)
    CUSTOM_DVE_SPECS[name] = op.spec
    _SUB_OPCODE_FOR_NAME[name] = row
    for ver in ("v3", "v4"):
        try:
            op.compile(ver)
        except ValueError as e:
            import re
            m = re.search(r'="([0-9a-f]{16})"', str(e))
            assert m, f"could not parse sha from: {e}"
            op.uops_sha[ver] = m.group(1)
    op.compile("v3")
    return op


def _ensure_custom_ops():
    """Register the fused DVE reduce ops (idempotent).

    TT_ADD_MINRED: accum_out = min(s0, min_k(Src0[k] + Src1[k]))
    TT_MIN_MINRED: accum_out = min(s0, min_k(min(Src0[k], Src1[k])))
      (feed a block's two halves as Src0/Src1 -> 2 elems per cycle)
    """
    from concourse.dve_spec import Spec, Src0, Src1, C0, minn

    add_op = _register_op(
        "TT_ADD_MINRED",
        Spec(body=Src0 + Src1, accum=minn, accum_init=C0,
             reference=_ref_add_minred))
    min_op = _register_op(
        "TT_MIN_MINRED",
        Spec(body=minn(Src0, Src1), accum=minn, accum_init=C0,
             reference=_ref_min_minred))
    return add_op, min_op


def _layout(counts):
    wmain = [max(1, min(int(n), WMAIN)) for n in counts]
    ov = [(c, int(n) - WMAIN) for c, n in enumerate(counts) if n > WMAIN]
    ovw = sum(w for _, w in ov)
    assert ovw <= 512, f"overflow region too wide: {ovw}"
    return tuple(wmain), tuple(ov)


def _build_program(Mc, wmain, ov, nterms=NTERMS, y_off=Y_OFF):
    import concourse.tile as tile
    from concourse import bacc, mybir

    cop, mop = _ensure_custom_ops()

    f32 = mybir.dt.float32
    f16 = mybir.dt.float16
    bf16 = mybir.dt.bfloat16
    AX = mybir.AxisListType.X
    OP = mybir.AluOpType

    Mt = Mc // P
    ovw = sum(w for _, w in ov)
    NB = NCLS + len(ov)
    N = int(sum(wmain)) + ovw
    moffs = np.concatenate([[0], np.cumsum(wmain)]).astype(int)
    off_blocks = list(range(y_off))          # ACT+Pool offload
    # offload fold tree needs width > 512
    off_blocks = [b for b in off_blocks if wmain[b] > 512]

    nc = bacc.Bacc("TRN2", target_bir_lowering=False, debug=False)

    v0d = [nc.dram_tensor(f"v0b{b}", [P, int(wmain[b])], bf16,
                          kind="ExternalInput") for b in range(NCLS)]
    v1d = [nc.dram_tensor(f"v1b{b}", [P, int(wmain[b])], bf16,
                          kind="ExternalInput") for b in range(NCLS)]
    v2d = {b: nc.dram_tensor(f"v2b{b}", [SQTERMS, int(wmain[b])], bf16,
                             kind="ExternalInput") for b in off_blocks}
    if ovw:
        v0od = nc.dram_tensor("v0ov", [P, ovw], bf16, kind="ExternalInput")
        v1od = nc.dram_tensor("v1ov", [P, ovw], bf16, kind="ExternalInput")
    uts = [
        nc.dram_tensor(f"u{k}t{t}", [P, Mc], bf16, kind="ExternalInput")
        for t in range(nterms) for k in range(2)
    ]
    sqbd = nc.dram_tensor("sqb16", [1, N], f16, kind="ExternalInput")
    sqa = nc.dram_tensor("sqa", [P, Mt], f32, kind="ExternalInput")
    vld = nc.dram_tensor("valid", [P, Mt], f32, kind="ExternalInput")
    pbig = nc.dram_tensor("posbig", [P, NB], f32, kind="ExternalInput")
    nbig = nc.dram_tensor("negbig", [P, NB], f32, kind="ExternalInput")
    out = nc.dram_tensor("out", [P, 2], f32, kind="ExternalOutput")

    with tile.TileContext(nc) as tc:
        with (
            tc.tile_pool(name="resident", bufs=1) as res,
            tc.tile_pool(name="psum", bufs=3, space="PSUM") as pp,
            tc.tile_pool(name="povf", bufs=2, space="PSUM") as po,
            tc.tile_pool(name="bmins", bufs=3) as bmp,
            tc.tile_pool(name="scrd", bufs=2) as scrp,
            tc.tile_pool(name="epi", bufs=12) as epi,
        ):
            # ---- PE warmup ------------------------------------------------
            wsrc = res.tile([P, 512], bf16, tag="wsrc")
            nc.vector.memset(wsrc[:], 0.0)
            wp = pp.tile([P, WMAIN], f32, tag="pblk", name="warm")
            for _ in range(26):
                nc.tensor.matmul(wp[:, 0:512], wsrc[:, 0:P], wsrc[:, :],
                                 start=True, stop=True)

            # ---- resident loads (keep the scalar queue free: ACT works) ---
            dma_engs = [nc.sync, nc.gpsimd]
            _dma_rr = [0]

            def dma(out_ap, in_ap):
                dma_engs[_dma_rr[0] % len(dma_engs)].dma_start(out=out_ap, in_=in_ap)
                _dma_rr[0] += 1

            utiles = []
            for i, ut in enumerate(uts):
                t = res.tile([P, Mc], bf16, tag=f"ut{i}", name=f"ut{i}")
                dma(t[:], ut[:, :])
                utiles.append(t)
            u2t = res.tile([32 + SQTERMS, Mc], bf16, tag="u2")
            nc.vector.memset(u2t[:], 1.0)

            sqat = res.tile([P, Mt], f32, tag="sqa")
            dma(sqat[:], sqa[:, :])
            vldt = res.tile([P, Mt], f32, tag="valid")
            dma(vldt[:], vld[:, :])
            pbigt = res.tile([P, NB], f32, tag="posbig")
            dma(pbigt[:], pbig[:, :])
            nbigt = res.tile([P, NB], f32, tag="negbig")
            dma(nbigt[:], nbig[:, :])

            # sqct: only cols of DIRECT blocks (+ovf) are read before the
            # last tile; load that slice early, the rest at fill end.
            sqct = res.tile([P, N], f16, tag="sqc16")
            direct_blocks = [b for b in range(NCLS) if b not in off_blocks]
            dstart = min([int(moffs[b]) for b in direct_blocks] or [N])

            v0ts, v1ts, v2ts = [], [], {}
            for b in range(NCLS):
                W = int(wmain[b])
                t0 = res.tile([P, W], bf16, tag=f"v0b{b}", name=f"v0b{b}")
                dma(t0[:], v0d[b][:, :])
                t1 = res.tile([P, W], bf16, tag=f"v1b{b}", name=f"v1b{b}")
                dma(t1[:], v1d[b][:, :])
                v0ts.append(t0)
                v1ts.append(t1)
                if b == 0:
                    nc.sync.dma_start(
                        out=sqct[:, dstart:N],
                        in_=sqbd[0:1, dstart:N].partition_broadcast(P))
                if b in off_blocks:
                    t2 = res.tile([32 + SQTERMS, W], bf16, tag=f"v2b{b}",
                                  name=f"v2b{b}")
                    for rp in (0, 32):
                        dma(t2[rp:rp + SQTERMS, :], v2d[b][:, :])
                    v2ts[b] = t2
            if ovw:
                ov0 = res.tile([P, ovw], bf16, tag="ov0")
                dma(ov0[:], v0od[:, :])
                ov1 = res.tile([P, ovw], bf16, tag="ov1")
                dma(ov1[:], v1od[:, :])

            # offload conversion tiles (padded to 1024 with fp16 "+inf")
            convs = {}
            for b in off_blocks:
                cb = res.tile([P, WMAIN], f16, tag=f"conv{b}")
                nc.gpsimd.memset(cb[:], PADV)
                convs[b] = cb

            if dstart > 0:
                nc.gpsimd.dma_start(
                    out=sqct[:, 0:dstart],
                    in_=sqbd[0:1, 0:dstart].partition_broadcast(P))

            num_sb = res.tile([P, Mt], f32, tag="num")
            pdists = res.tile([P, Mt], f32, tag="pdists")
            ndists = res.tile([P, Mt], f32, tag="ndists")
            pmins = res.tile([P, Mt], f32, tag="pmins")
            nmins = res.tile([P, Mt], f32, tag="nmins")
            out_sb = res.tile([P, 2], f32, tag="out")

            # ---- main loop ------------------------------------------------
            for mt in range(Mt):
                ms = slice(mt * P, (mt + 1) * P)
                bmins = bmp.tile([P, NB], f32, tag="bm")
                items = list(range(NCLS)) + (["ovf"] if ovw else [])
                for item in items:
                  if item != "ovf":
                    b = item
                    W = int(wmain[b])
                    c0 = int(moffs[b])
                    ptile = pp.tile([P, W], f32, tag="pblk", name="pblk")
                    segs = [(i, min(512, W - i)) for i in range(0, W, 512)]
                    stats = []
                    for t in range(nterms):
                        stats.append((utiles[2 * t], v0ts[b]))
                        stats.append((utiles[2 * t + 1], v1ts[b]))
                    is_off = (b in off_blocks) and (mt < Mt - 1)
                    for ti, (ut, vt) in enumerate(stats):
                        last_stat = (ti == len(stats) - 1) and not is_off
                        for i, s in segs:
                            cs = slice(i, i + s)
                            nc.tensor.matmul(
                                ptile[:, cs], ut[:, ms], vt[:, cs],
                                start=(ti == 0), stop=last_stat,
                            )
                    if is_off:
                        # K=3 sq channel rides the PE for offload blocks
                        for si, (i, s) in enumerate(segs):
                            cs = slice(i, i + s)
                            rp = 32 * (si % 2)
                            nc.tensor.matmul(
                                ptile[:, cs],
                                u2t[rp:rp + SQTERMS, ms],
                                v2ts[b][rp:rp + SQTERMS, cs],
                                start=False, stop=True,
                                tile_position=(rp, 0),
                            )
                        cb = convs[b]
                        nc.scalar.copy(cb[:, 0:W], ptile[:, :])
                        # fused pair-min reduce: both halves in one 512-pass
                        fsc = scrp.tile([P, 512], f16, tag="fold")
                        nc.vector._custom_dve(
                            mop, out=fsc[:, 0:512], in0=cb[:, 0:512],
                            in1=cb[:, 512:1024], s0=BIGM,
                            accum_out=bmins[:, b:b + 1])
                    else:
                        # fused add+min against ±(sq-256) on the DVE
                        scrd = scrp.tile([P, W], f16, tag="scrd")
                        nc.vector._custom_dve(
                            cop, out=scrd[:, :], in0=ptile[:, :],
                            in1=sqct[:, c0:c0 + W], s0=BIGM,
                            accum_out=bmins[:, b:b + 1])

                  else:
                    otile = po.tile([P, ovw], f32, tag="ovf", name="ovf")
                    ostats = []
                    for t in range(nterms):
                        ostats.append((utiles[2 * t], ov0))
                        ostats.append((utiles[2 * t + 1], ov1))
                    for ti, (ut, vt) in enumerate(ostats):
                        nc.tensor.matmul(
                            otile[:, :], ut[:, ms], vt[:, :],
                            start=(ti == 0), stop=(ti == len(ostats) - 1),
                        )
                    oo2 = 0
                    obase = int(moffs[NCLS])
                    for k, (cls, w) in enumerate(ov):
                        scro = scrp.tile([P, max(ovw, 1)], f16, tag="scro")
                        nc.vector._custom_dve(
                            cop, out=scro[:, 0:w], in0=otile[:, oo2:oo2 + w],
                            in1=sqct[:, obase + oo2:obase + oo2 + w], s0=BIGM,
                            accum_out=bmins[:, NCLS + k:NCLS + k + 1])
                        oo2 += w

                # ---- epilogue for this anchor tile (fused, DVE only) ------
                # pmins/nmins = min(bmins + BIGM-mask) in one custom op each
                t8a = epi.tile([P, NB], f32, tag="t8a")
                nc.vector._custom_dve(
                    cop, out=t8a[:], in0=bmins[:], in1=pbigt[:], s0=BIGM,
                    accum_out=pmins[:, mt:mt + 1])
                t8b = epi.tile([P, NB], f32, tag="t8b")
                nc.vector._custom_dve(
                    cop, out=t8b[:], in0=bmins[:], in1=nbigt[:], s0=BIGM,
                    accum_out=nmins[:, mt:mt + 1])

            # ---- deferred epilogue (one batched pass) --------------------
            # pos_d2 = relu(sqa256 - pmins), neg_d2 = relu(sqa256 + nmins)
            pd2 = epi.tile([P, Mt], f32, tag="pd2")
            nc.vector.scalar_tensor_tensor(
                pd2[:], in0=pmins[:], scalar=-1.0, in1=sqat[:],
                op0=OP.mult, op1=OP.add)
            nc.vector.tensor_scalar_max(pd2[:], pd2[:], 0.0)
            nd2 = epi.tile([P, Mt], f32, tag="nd2")
            nc.vector.scalar_tensor_tensor(
                nd2[:], in0=nmins[:], scalar=1.0, in1=sqat[:],
                op0=OP.mult, op1=OP.add)
            nc.vector.tensor_scalar_max(nd2[:], nd2[:], 0.0)
            nc.scalar.sqrt(pdists[:], pd2[:])
            nc.scalar.sqrt(ndists[:], nd2[:])

            per = epi.tile([P, Mt], f32, tag="per")
            nc.vector.scalar_tensor_tensor(
                per[:], in0=pdists[:], scalar=MARGIN, in1=ndists[:],
                op0=OP.add, op1=OP.subtract,
            )
            perr = epi.tile([P, Mt], f32, tag="perr")
            nc.vector.tensor_scalar_max(perr[:], per[:], 0.0)
            nc.vector.tensor_tensor(num_sb[:], perr[:], vldt[:], op=OP.mult)

            nc.vector.tensor_reduce(out_sb[:, 0:1], num_sb[:], axis=AX, op=OP.add)
            nc.vector.tensor_reduce(out_sb[:, 1:2], vldt[:], axis=AX, op=OP.add)
            nc.sync.dma_start(out=out[:, :], in_=out_sb[:])

    nc.compile()
    return nc


def _bf16_terms(x, nterms):
    terms = []
    r = x.astype(np.float32)
    for _ in range(nterms):
        h = r.astype(BF16)
        terms.append(h)
        r = r - h.astype(np.float32)
    return terms


def _prepare_inputs(emb, lab, nterms=NTERMS, y_off=Y_OFF):
    B = emb.shape[0]
    assert emb.shape[1] == D
    counts = np.bincount(lab, minlength=NCLS).astype(int)
    assert counts.sum() == B

    order = np.argsort(lab, kind="stable")
    cstart = np.concatenate([[0], np.cumsum(counts)]).astype(int)

    wmain, ov = _layout(counts)
    ovw = sum(w for _, w in ov)
    NB = NCLS + len(ov)
    Mc = int(((max(1, counts.max()) + P - 1) // P) * P)
    Mt = Mc // P
    N = int(sum(wmain)) + ovw
    off_blocks = [b for b in range(y_off) if wmain[b] > 512]

    sq = np.einsum("ij,ij->i", emb, emb, dtype=np.float32)

    colidx = np.empty(N, dtype=np.int64)
    own_ranges = {c: [] for c in range(NCLS)}
    off = 0
    for c in range(NCLS):
        idx = order[cstart[c]:cstart[c + 1]][:wmain[c]]
        if len(idx) == 0:
            idx = order[0:1]
        w = wmain[c]
        colidx[off:off + w] = idx
        own_ranges[c].append((off, w))
        off += w
    for cls, w in ov:
        idx = order[cstart[cls] + WMAIN:cstart[cls + 1]]
        assert len(idx) == w
        colidx[off:off + w] = idx
        own_ranges[cls].append((off, w))
        off += w

    Vg = np.ascontiguousarray(emb[colidx].T).astype(BF16)     # [256, N]
    sqo = sq - np.float32(SQOFF)                              # sq - 256
    sq_terms = _bf16_terms(sqo, SQTERMS)
    sqf_t = np.stack([t[colidx] for t in sq_terms])           # [SQTERMS, N]

    u_full = _bf16_terms(-2.0 * emb, nterms)

    bm_cls = list(range(NCLS)) + [cls for cls, _ in ov]

    in_maps = []
    for c in range(NCLS):
        aidx = order[cstart[c]:cstart[c + 1]]
        if len(aidx) == 0:
            aidx = order[0:1]
        npad = Mc - len(aidx)
        pad = np.full(npad, aidx[0], dtype=np.int64)
        aidx_p = np.concatenate([aidx, pad])

        real = np.zeros(Mc, dtype=np.float32)
        real[: min(len(aidx), Mc)] = 1.0
        cls_valid = 1.0 if (2 <= counts[c] <= B - 1) else 0.0
        valid = (real * cls_valid).reshape(Mt, P).T.copy()

        sqa_t = (sq[aidx_p] + np.float32(SQOFF)).reshape(Mt, P).T.copy()

        s = np.ones(N, dtype=np.float32)
        for o, w in own_ranges[c]:
            s[o:o + w] = -1.0
        sb = s.astype(BF16)

        posbig = np.zeros((P, NB), dtype=np.float32)
        negbig = np.zeros((P, NB), dtype=np.float32)
        for j, bc in enumerate(bm_cls):
            if bc == c:
                negbig[:, j] = BIGM
            else:
                posbig[:, j] = BIGM

        vv0 = Vg[0:128] * sb
        vv1 = Vg[128:256] * sb
        vv2 = sqf_t * sb
        sqb16 = (sqo[colidx] * s).astype(np.float16).reshape(1, N)
        im = {
            "sqa": sqa_t,
            "valid": valid,
            "posbig": posbig,
            "negbig": negbig,
            "sqb16": sqb16,
        }
        off2 = 0
        for b in range(NCLS):
            w = wmain[b]
            im[f"v0b{b}"] = np.ascontiguousarray(vv0[:, off2:off2 + w])
            im[f"v1b{b}"] = np.ascontiguousarray(vv1[:, off2:off2 + w])
            if b in off_blocks:
                im[f"v2b{b}"] = np.ascontiguousarray(vv2[:, off2:off2 + w])
            off2 += w
        if ovw:
            im["v0ov"] = np.ascontiguousarray(vv0[:, off2:])
            im["v1ov"] = np.ascontiguousarray(vv1[:, off2:])
        for t in range(nterms):
            ut = u_full[t][aidx_p]
            im[f"u0t{t}"] = np.ascontiguousarray(ut[:, 0:128].T)
            im[f"u1t{t}"] = np.ascontiguousarray(ut[:, 128:256].T)
        in_maps.append(im)

    meta = dict(Mc=Mc, wmain=wmain, ov=ov, Mt=Mt, N=N)
    return in_maps, meta


_PROGRAM_CACHE = {}


def _get_program(Mc, wmain, ov):
    key = (Mc, wmain, ov, NTERMS, Y_OFF, "v12")
    if key not in _PROGRAM_CACHE:
        _PROGRAM_CACHE[key] = _build_program(Mc, wmain, ov, NTERMS, Y_OFF)
    return _PROGRAM_CACHE[key]


def _combine(results):
    num = 0.0
    den = 0.0
    for r in results:
        o = np.asarray(r["out"], dtype=np.float64)
        num += o[:, 0].sum()
        den += o[:, 1].sum()
    return np.float32(num / max(den, 1.0))


def _setup_trace_hook():
    import sys
    import types
    try:
        from antenv.axon_hooks import get_axon_ntff_profile_hook  # noqa: F401
        return
    except ImportError:
        pass
    import antenv
    from trn_agent_boot.trn_boot import _ntff_profile_via_ctypes

    mod = types.ModuleType("antenv.axon_hooks")
    state = {"h": None}
    mod.set_axon_ntff_profile_hook = lambda h: state.__setitem__("h", h)
    mod.get_axon_ntff_profile_hook = lambda: state["h"]
    sys.modules["antenv.axon_hooks"] = mod
    antenv.axon_hooks = mod
    mod.set_axon_ntff_profile_hook(
        _ntff_profile_via_ctypes("/opt/axon/libaxon_pjrt.so")
    )


def kernel(embeddings, labels, _trace=False):
    emb = np.ascontiguousarray(np.asarray(embeddings, dtype=np.float32))
    lab = np.asarray(labels).astype(np.int64).ravel()

    in_maps, meta = _prepare_inputs(emb, lab)
    nc = _get_program(meta["Mc"], meta["wmain"], meta["ov"])

    from concourse.bass_utils import run_bass_kernel_spmd

    if _trace:
        _setup_trace_hook()
        import concourse.bass_utils as _bu
        _bu.upload_artifacts = lambda tmpdir: tmpdir

    res = run_bass_kernel_spmd(
        nc, in_maps, core_ids=list(range(NCORES)), trace=bool(_trace),
    )
    loss = _combine(res.results)
    if _trace:
        return loss, res
    return loss

